# revision 18
# baseline (speedup 1.0000x reference)
"""Trainium2 Bass kernel for nn_LstmCrf: bidirectional LSTM + CRF log-partition.

Contract: kernel(**inputs) takes the FULL unsharded inputs and returns the FULL
output logZ [128] f32. Internally shards the batch (128 rows) across 8
NeuronCores (16 rows each), runs one SPMD Bass/Tile program, and concatenates
the per-core results.

Problem shapes (hardcoded): B=128, T=512, V=50000, E=100, U=128, K=32.

v2 design (vs lockstep v1 @2.21us/step): the fwd and bwd LSTM scans run as two
DECOUPLED dependency chains, interleaved so each engine alternates between the
chains and the ~1.6us per-step chain latency of one chain hides behind the
other.  Per chain-step: 4 x-proj MMs are emitted one step ahead (fill PE idle),
4 h-proj MMs -> sigmoid ACT [128,64] -> 3 fused DVE ops for the cell update
(layout trick: sg tile [128,80] = [i f o g | c_prev] makes (si|sf)*(sgg|c_prev)
a single tensor_tensor) -> tanh ACT [128,16] -> 1 DVE h-mult straight into
h_all.  ACT queue order per step is [sig_f, sig_b, tanh_f, tanh_b].

Emissions: em_e = exp(em + bias - delta) bf16 as before.  CRF: exp-domain
meet-in-the-middle DP with BF16 transition matrices (fp32 lhsT costs 2 HW
matmuls per logical matmul on the PE; bf16 costs 1).
"""
import sys
from contextlib import ExitStack

import numpy as np

for p in ("/opt/trn_rl_repo", "/root/.axon_site/_ro/trn_rl_repo"):
    if p not in sys.path:
        sys.path.append(p)

import ml_dtypes

NPBF16 = ml_dtypes.bfloat16

B, T = 128, 512
V, E, U, K = 50000, 100, 128, 32
NCORES = 8
BL = B // NCORES          # 16 rows per core
EA = 104                  # padded embedding dim
G4 = 4 * U
DELTA = float(np.log(K))


def _build_program(T=T):
    import concourse.bacc as bacc
    import concourse.bass as bass
    import concourse.mybir as mybir
    import concourse.tile as tile

    F32 = mybir.dt.float32
    BF16 = mybir.dt.bfloat16
    I32 = mybir.dt.int32
    AF = mybir.ActivationFunctionType
    ALU = mybir.AluOpType

    NCH = 8                   # chunks per direction
    WARM = 8                  # warmup steps per chunk
    NP_ = T // NCH + WARM     # 80 lockstep periods
    CB = NCH * BL             # 128 cols per period block
    MID = T // 2

    nc = bacc.Bacc(None, target_bir_lowering=False, debug=False)

    tok_f = nc.dram_tensor("tok_f", [128, NP_], I32, kind="ExternalInput")
    tok_b = nc.dram_tensor("tok_b", [128, NP_], I32, kind="ExternalInput")
    emb = nc.dram_tensor("emb", [V, 128], BF16, kind="ExternalInput")
    wk_f = nc.dram_tensor("wk_f", [128, G4], BF16, kind="ExternalInput")
    wk_b = nc.dram_tensor("wk_b", [128, G4], BF16, kind="ExternalInput")
    wr_f = nc.dram_tensor("wr_f", [U, G4], BF16, kind="ExternalInput")
    wr_b = nc.dram_tensor("wr_b", [U, G4], BF16, kind="ExternalInput")
    ck_f = nc.dram_tensor("ck_f", [U, K], BF16, kind="ExternalInput")
    ck_b = nc.dram_tensor("ck_b", [U, K], BF16, kind="ExternalInput")
    ae = nc.dram_tensor("ae", [K, K], BF16, kind="ExternalInput")
    aet = nc.dram_tensor("aet", [K, K], BF16, kind="ExternalInput")
    embias = nc.dram_tensor("embias", [K, 1], F32, kind="ExternalInput")
    out = nc.dram_tensor("out", [1, BL], F32, kind="ExternalOutput")

    def block_order(nblk):
        order = []
        lo, hi = 0, nblk - 1
        while lo <= hi:
            order.append(lo)
            if hi != lo:
                order.append(hi)
            lo += 1
            hi -= 1
        return order

    with tile.TileContext(nc) as tc, ExitStack() as ctx:
        P = ctx.enter_context(tc.tile_pool(name="persist", bufs=1))
        tokf_t = P.tile([128, NP_], I32, tag="tokf")
        tokb_t = P.tile([128, NP_], I32, tag="tokb")
        wkf_t = P.tile([128, G4], BF16, tag="wkf")
        wkb_t = P.tile([128, G4], BF16, tag="wkb")
        wrf_t = P.tile([U, G4], BF16, tag="wrf")
        wrb_t = P.tile([U, G4], BF16, tag="wrb")
        ckf_t = P.tile([U, K], BF16, tag="ckf")
        ckb_t = P.tile([U, K], BF16, tag="ckb")
        ae_t = P.tile([K, K], BF16, tag="ae")
        aet_t = P.tile([K, K], BF16, tag="aet")
        embias_t = P.tile([K, 1], F32, tag="embias")
        xTf = P.tile([128, NP_ * CB], BF16, tag="xTf")
        xTb = P.tile([128, NP_ * CB], BF16, tag="xTb")
        h_f = P.tile([U, NP_ * CB], BF16, tag="hf")
        h_b = P.tile([U, (NP_ + WARM) * CB], BF16, tag="hb")
        em_e = P.tile([K, T * BL], BF16, tag="eme")
        ones_t = P.tile([K, 1], F32, tag="ones")
        neg1_t = P.tile([128, 1], F32, tag="neg1")

        nc.sync.dma_start(tokf_t[:], tok_f[:])
        nc.sync.dma_start(tokb_t[:], tok_b[:])
        nc.sync.dma_start(wkf_t[:], wk_f[:])
        nc.sync.dma_start(wkb_t[:], wk_b[:])
        nc.sync.dma_start(wrf_t[:], wr_f[:])
        nc.sync.dma_start(wrb_t[:], wr_b[:])
        nc.sync.dma_start(ckf_t[:], ck_f[:])
        nc.sync.dma_start(ckb_t[:], ck_b[:])
        nc.sync.dma_start(ae_t[:], ae[:])
        nc.sync.dma_start(aet_t[:], aet[:])
        nc.sync.dma_start(embias_t[:], embias[:])
        nc.vector.memset(ones_t[:], 1.0)
        nc.vector.memset(neg1_t[:], -1.0)

        wk_ts = (wkf_t, wkb_t)
        wr_ts = (wrf_t, wrb_t)
        tok_ts = (tokf_t, tokb_t)
        xT_ts = (xTf, xTb)
        h_ts = (h_f, h_b)

        with ExitStack() as sctx:
            gat = sctx.enter_context(tc.tile_pool(name="gat", bufs=4))
            zpool = tuple(
                sctx.enter_context(tc.tile_pool(name=f"z{i}", bufs=1, space="PSUM"))
                for i in range(2))
            sgpool = tuple(
                sctx.enter_context(tc.tile_pool(name=f"sg{i}", bufs=3))
                for i in range(2))
            scrpool = tuple(
                sctx.enter_context(tc.tile_pool(name=f"scr{i}", bufs=2))
                for i in range(2))
            thpool = tuple(
                sctx.enter_context(tc.tile_pool(name=f"th{i}", bufs=2))
                for i in range(2))

            def emit_block(d, s):
                g = gat.tile([128, 128], BF16, tag="g", name="g")
                nc.gpsimd.indirect_dma_start(
                    out=g[:],
                    out_offset=None,
                    in_=emb[:],
                    in_offset=bass.IndirectOffsetOnAxis(
                        ap=tok_ts[d][:, s:s + 1], axis=0),
                )
                nc.sync.dma_start_transpose(
                    xT_ts[d][:, s * CB:(s + 1) * CB], g[:])

            # h block position: fwd writes block s; bwd writes block
            # (NP_ + WARM - 1) - s so that real blocks [WARM, NP_) of h_f and
            # h_b are time-aligned (bwd chunk slots are host-relabeled).
            def hpos(d, s):
                return s if d == 0 else (NP_ + WARM - 1) - s

            gi_next = [0, 0]
            for s in range(4):
                emit_block(0, s)
                emit_block(1, s)
            gfetched = 4

            sg_cur = [None, None]
            z_cur = [None, None]
            th = [None, None]
            for s in range(NP_):
                while gfetched < min(NP_, s + 4):
                    emit_block(0, gfetched)
                    emit_block(1, gfetched)
                    gfetched += 1
                # chunk-0 boundary reset: before the s=WARM h-MMs, zero the
                # exact-start chunk's h and set its cell state to zero
                # (chat = 1/2).  fwd exact chunk is slot 0; bwd is slot NCH-1.
                if s == WARM:
                    nc.vector.memset(
                        h_f[:, (WARM - 1) * CB:(WARM - 1) * CB + BL], 0.0)
                    qb = (NCH - 1) * BL
                    nc.vector.memset(
                        h_b[:, hpos(1, WARM - 1) * CB + qb:
                            hpos(1, WARM - 1) * CB + qb + BL], 0.0)
                    nc.vector.memset(sg_cur[0][:, 4 * CB:4 * CB + BL], 0.5)
                    nc.vector.memset(sg_cur[1][:, 4 * CB + qb:5 * CB], 0.5)
                # PE: x-MMs then h-MMs per chain.
                for d in (0, 1):
                    z_cur[d] = zpool[d].tile([128, 4 * CB], F32, tag="z",
                                             name=f"z{d}")
                    xs = xT_ts[d][:, s * CB:(s + 1) * CB]
                    for gi in range(4):
                        nc.tensor.matmul(
                            z_cur[d][:, gi * CB:(gi + 1) * CB],
                            wk_ts[d][:, gi * U:(gi + 1) * U],
                            xs,
                            start=(gi == 0),
                            stop=(s == 0 and gi == 3),
                        )
                    if s > 0:
                        hs = h_ts[d][:, hpos(d, s - 1) * CB:
                                     (hpos(d, s - 1) + 1) * CB]
                        for gi in range(4):
                            nc.tensor.matmul(
                                z_cur[d][:, gi * CB:(gi + 1) * CB],
                                wr_ts[d][:, gi * U:(gi + 1) * U],
                                hs,
                                start=False,
                                stop=(gi == 3),
                            )
                # ACT: sigmoids.
                for d in (0, 1):
                    if s == 0:
                        sg_cur[d] = sgpool[d].tile([128, 5 * CB], BF16,
                                                   tag="sg", name=f"sg{d}")
                    nc.scalar.activation(sg_cur[d][:, 0:4 * CB], z_cur[d][:],
                                         AF.Sigmoid)
                # DVE: cell update (chat = c/2 + 1/2 storage).
                sg_next = [None, None]
                for d in (0, 1):
                    sg_next[d] = sgpool[d].tile([128, 5 * CB], BF16, tag="sg",
                                                name=f"sg{d}")
                    sg = sg_cur[d]
                    cdst = sg_next[d][:, 4 * CB:5 * CB]
                    if s == 0:
                        a0 = scrpool[d].tile([128, CB], BF16, tag="ab",
                                             name=f"ab{d}")
                        nc.vector.scalar_tensor_tensor(
                            a0[:], sg[:, 3 * CB:4 * CB], 0.5, sg[:, 0:CB],
                            ALU.subtract, ALU.mult)
                        nc.vector.tensor_scalar(cdst, a0[:], 0.5, None, ALU.add)
                    else:
                        ab = scrpool[d].tile([128, 2 * CB], BF16, tag="ab",
                                             name=f"ab{d}")
                        nc.vector.scalar_tensor_tensor(
                            ab[:], sg[:, 3 * CB:5 * CB], 0.5, sg[:, 0:2 * CB],
                            ALU.subtract, ALU.mult)
                        nc.vector.scalar_tensor_tensor(
                            cdst, ab[:, 0:CB], 0.5, ab[:, CB:2 * CB],
                            ALU.add, ALU.add)
                # ACT: tanh(c) = tanh(2*chat - 1).
                for d in (0, 1):
                    th[d] = thpool[d].tile([128, CB], BF16, tag="th",
                                           name=f"th{d}")
                    nc.scalar.activation(th[d][:], sg_next[d][:, 4 * CB:5 * CB],
                                         AF.Tanh, bias=neg1_t[:], scale=2.0)
                # DVE: h = so * tanh(c).
                for d in (0, 1):
                    nc.vector.tensor_tensor(
                        h_ts[d][:, hpos(d, s) * CB:(hpos(d, s) + 1) * CB],
                        sg_cur[d][:, 2 * CB:3 * CB], th[d][:], ALU.mult)
                    sg_cur[d] = sg_next[d]

        # keep the exp/ln table phase strictly after the sigmoid/tanh phase
        tc.no_sync_barrier()

        EMC = 512
        with (
            tc.tile_pool(name="emps", bufs=4, space="PSUM") as emps,
            tc.tile_pool(name="crf", bufs=4) as crf,
            tc.tile_pool(name="crfps", bufs=2, space="PSUM") as crfps,
        ):
            nchunk = T * BL // EMC
            emorder = []
            lo, hi = 0, nchunk - 1
            while lo <= hi:
                emorder.append(lo)
                if hi != lo:
                    emorder.append(hi)
                lo += 1
                hi -= 1
            RB = WARM * CB  # start of the real (non-warmup) region
            for ch in emorder:
                ep = emps.tile([K, EMC], F32, tag="ep")
                nc.tensor.matmul(ep[:], ckf_t[:],
                                 h_f[:, RB + ch * EMC:RB + (ch + 1) * EMC],
                                 start=True, stop=False)
                nc.tensor.matmul(ep[:], ckb_t[:],
                                 h_b[:, RB + ch * EMC:RB + (ch + 1) * EMC],
                                 start=False, stop=True)
                nc.scalar.activation(em_e[:, ch * EMC:(ch + 1) * EMC], ep[:],
                                     AF.Exp, bias=embias_t[:], scale=1.0)

            def ecol(tau):
                return (tau % (T // NCH)) * CB + (tau // (T // NCH)) * BL

            a_cur = crf.tile([K, BL], BF16, tag="a")
            nc.vector.tensor_copy(a_cur[:], em_e[:, ecol(0):ecol(0) + BL])
            b_cur = crf.tile([K, BL], BF16, tag="b")
            nc.vector.tensor_copy(b_cur[:], em_e[:, ecol(T - 1):ecol(T - 1) + BL])

            for s in range(1, MID + 1):
                aps = crfps.tile([K, BL], F32, tag="aps")
                nc.tensor.matmul(aps[:], ae_t[:], a_cur[:], start=True, stop=True)
                a_new = crf.tile([K, BL], BF16, tag="a")
                nc.vector.tensor_tensor(a_new[:], aps[:],
                                        em_e[:, ecol(s):ecol(s) + BL], ALU.mult)
                a_cur = a_new

                if s <= MID - 1:
                    t_b = T - 1 - s
                    bps = crfps.tile([K, BL], F32, tag="bps")
                    nc.tensor.matmul(bps[:], aet_t[:], b_cur[:], start=True, stop=True)
                    b_new = crf.tile([K, BL], BF16, tag="b")
                    if t_b == MID:
                        nc.vector.tensor_copy(b_new[:], bps[:])
                    else:
                        nc.vector.tensor_tensor(b_new[:], bps[:],
                                                em_e[:, ecol(t_b):ecol(t_b) + BL],
                                                ALU.mult)
                    b_cur = b_new

            prod = crf.tile([K, BL], F32, tag="prod")
            nc.vector.tensor_tensor(prod[:], a_cur[:], b_cur[:], ALU.mult)
            sps = crfps.tile([1, BL], F32, tag="aps")
            nc.tensor.matmul(sps[:], ones_t[:], prod[:], start=True, stop=True)
            logz = crf.tile([1, BL], F32, tag="logz")
            nc.scalar.activation(logz[:], sps[:], AF.Ln)
            logz2 = crf.tile([1, BL], F32, tag="logz2")
            nc.vector.tensor_scalar(logz2[:], logz[:], float(T * DELTA), None, ALU.add)
            nc.sync.dma_start(out[:], logz2[:])

    nc.compile()
    return nc


def _gate_permute(w):
    """Reorder gate blocks from reference (i,f,g,o) to kernel (i,f,o,g) and
    pre-double the g block so tanh(g) = 2*sigmoid(2g)-1 needs only sigmoid."""
    i, f, g, o = np.split(w, 4, axis=-1)
    return np.concatenate([i, f, o, 2.0 * g], axis=-1)


def _stage(tokens, emb, Wk_f, Wr_f, b_f, Wk_b, Wr_b, b_b, crf_kernel, crf_bias,
           trans):
    """Host staging: build the per-core input maps."""
    emb_aug = np.concatenate(
        [emb, np.ones((V, 1), np.float32), np.zeros((V, 128 - E - 1), np.float32)], 1)
    wk_aug_f = np.concatenate([Wk_f, b_f[None], np.zeros((128 - E - 1, G4), np.float32)], 0)
    wk_aug_b = np.concatenate([Wk_b, b_b[None], np.zeros((128 - E - 1, G4), np.float32)], 0)
    Ae = np.exp(trans).astype(np.float32)

    shared = {
        "emb": emb_aug.astype(NPBF16),
        "wk_f": np.ascontiguousarray(_gate_permute(wk_aug_f)).astype(NPBF16),
        "wk_b": np.ascontiguousarray(_gate_permute(wk_aug_b)).astype(NPBF16),
        "wr_f": np.ascontiguousarray(_gate_permute(Wr_f)).astype(NPBF16),
        "wr_b": np.ascontiguousarray(_gate_permute(Wr_b)).astype(NPBF16),
        "ck_f": np.ascontiguousarray(crf_kernel[:U]).astype(NPBF16),
        "ck_b": np.ascontiguousarray(crf_kernel[U:]).astype(NPBF16),
        "ae": np.ascontiguousarray(Ae).astype(NPBF16),
        "aet": np.ascontiguousarray(Ae.T).astype(NPBF16),
        "embias": (crf_bias - DELTA).astype(np.float32).reshape(K, 1),
    }

    NCH, WARM = 8, 8
    NP_ = T // NCH + WARM
    CL = T // NCH
    ss = np.arange(NP_)[:, None]
    jj = np.arange(NCH)[None, :]
    tf = np.clip(CL * jj - WARM + ss, 0, T - 1)           # [NP_, NCH] fwd times
    tb = np.clip(CL - 1 + WARM + CL * jj - ss, 0, T - 1)  # bwd (slot-relabeled)
    in_maps = []
    for c in range(NCORES):
        tc_ = tokens[c * BL:(c + 1) * BL].astype(np.int32)  # [16, T]
        tok_f = tc_[:, tf].transpose(2, 0, 1).reshape(NCH * BL, NP_)
        tok_b = tc_[:, tb].transpose(2, 0, 1).reshape(NCH * BL, NP_)
        in_maps.append({"tok_f": np.ascontiguousarray(tok_f),
                        "tok_b": np.ascontiguousarray(tok_b), **shared})
    return in_maps


_PROGRAM_CACHE = {}


def kernel(tokens, emb, Wk_f, Wr_f, b_f, Wk_b, Wr_b, b_b, crf_kernel, crf_bias, trans):
    from concourse.bass_utils import run_bass_kernel_spmd

    tokens = np.asarray(tokens)
    emb = np.asarray(emb, dtype=np.float32)
    Wk_f = np.asarray(Wk_f, np.float32); Wr_f = np.asarray(Wr_f, np.float32)
    Wk_b = np.asarray(Wk_b, np.float32); Wr_b = np.asarray(Wr_b, np.float32)
    b_f = np.asarray(b_f, np.float32); b_b = np.asarray(b_b, np.float32)
    crf_kernel = np.asarray(crf_kernel, np.float32)
    crf_bias = np.asarray(crf_bias, np.float32)
    trans = np.asarray(trans, np.float32)

    if "nc" not in _PROGRAM_CACHE:
        _PROGRAM_CACHE["nc"] = _build_program()
    nc = _PROGRAM_CACHE["nc"]

    in_maps = _stage(tokens, emb, Wk_f, Wr_f, b_f, Wk_b, Wr_b, b_b,
                     crf_kernel, crf_bias, trans)
    res = run_bass_kernel_spmd(nc, in_maps, core_ids=list(range(NCORES)))
    outs = [res.results[c]["out"].reshape(BL).astype(np.float32) for c in range(NCORES)]
    return np.concatenate(outs, axis=0)


# revision 19
# speedup vs baseline: 1.4065x; 1.4065x over previous
"""Trainium2 Bass kernel for nn_LstmCrf: bidirectional LSTM + CRF log-partition.

Contract: kernel(**inputs) takes the FULL unsharded inputs and returns the FULL
output logZ [128] f32. Internally shards the batch (128 rows) across 8
NeuronCores (16 rows each), runs one SPMD Bass/Tile program, and concatenates
the per-core results.

Problem shapes (hardcoded): B=128, T=512, V=50000, E=100, U=128, K=32.

v2 design (vs lockstep v1 @2.21us/step): the fwd and bwd LSTM scans run as two
DECOUPLED dependency chains, interleaved so each engine alternates between the
chains and the ~1.6us per-step chain latency of one chain hides behind the
other.  Per chain-step: 4 x-proj MMs are emitted one step ahead (fill PE idle),
4 h-proj MMs -> sigmoid ACT [128,64] -> 3 fused DVE ops for the cell update
(layout trick: sg tile [128,80] = [i f o g | c_prev] makes (si|sf)*(sgg|c_prev)
a single tensor_tensor) -> tanh ACT [128,16] -> 1 DVE h-mult straight into
h_all.  ACT queue order per step is [sig_f, sig_b, tanh_f, tanh_b].

Emissions: em_e = exp(em + bias - delta) bf16 as before.  CRF: exp-domain
meet-in-the-middle DP with BF16 transition matrices (fp32 lhsT costs 2 HW
matmuls per logical matmul on the PE; bf16 costs 1).
"""
import sys
from contextlib import ExitStack

import numpy as np

for p in ("/opt/trn_rl_repo", "/root/.axon_site/_ro/trn_rl_repo"):
    if p not in sys.path:
        sys.path.append(p)

import ml_dtypes

NPBF16 = ml_dtypes.bfloat16

B, T = 128, 512
V, E, U, K = 50000, 100, 128, 32
NCORES = 8
BL = B // NCORES          # 16 rows per core
EA = 104                  # padded embedding dim
G4 = 4 * U
DELTA = float(np.log(K))


def _build_program(T=T):
    import concourse.bacc as bacc
    import concourse.bass as bass
    import concourse.mybir as mybir
    import concourse.tile as tile

    F32 = mybir.dt.float32
    BF16 = mybir.dt.bfloat16
    I32 = mybir.dt.int32
    AF = mybir.ActivationFunctionType
    ALU = mybir.AluOpType

    NCH = 8                   # chunks per direction
    WARM = 8                  # warmup steps per chunk
    NP_ = T // NCH + WARM     # 80 lockstep periods
    CB = NCH * BL             # 128 cols per period block
    MID = T // 2

    nc = bacc.Bacc(None, target_bir_lowering=False, debug=False)

    tok_f = nc.dram_tensor("tok_f", [128, NP_], I32, kind="ExternalInput")
    tok_b = nc.dram_tensor("tok_b", [128, NP_], I32, kind="ExternalInput")
    emb = nc.dram_tensor("emb", [V, EA], F32, kind="ExternalInput")
    wk_f = nc.dram_tensor("wk_f", [EA, G4], BF16, kind="ExternalInput")
    wk_b = nc.dram_tensor("wk_b", [EA, G4], BF16, kind="ExternalInput")
    wr_f = nc.dram_tensor("wr_f", [U, G4], BF16, kind="ExternalInput")
    wr_b = nc.dram_tensor("wr_b", [U, G4], BF16, kind="ExternalInput")
    ck_f = nc.dram_tensor("ck_f", [U, K], BF16, kind="ExternalInput")
    ck_b = nc.dram_tensor("ck_b", [U, K], BF16, kind="ExternalInput")
    ae = nc.dram_tensor("ae", [K, K], BF16, kind="ExternalInput")
    aet = nc.dram_tensor("aet", [K, K], BF16, kind="ExternalInput")
    embias = nc.dram_tensor("embias", [K, 1], F32, kind="ExternalInput")
    ident = nc.dram_tensor("ident", [128, 128], F32, kind="ExternalInput")
    out = nc.dram_tensor("out", [1, BL], F32, kind="ExternalOutput")

    def block_order(nblk):
        order = []
        lo, hi = 0, nblk - 1
        while lo <= hi:
            order.append(lo)
            if hi != lo:
                order.append(hi)
            lo += 1
            hi -= 1
        return order

    with tile.TileContext(nc) as tc, ExitStack() as ctx:
        P = ctx.enter_context(tc.tile_pool(name="persist", bufs=1))
        tokf_t = P.tile([128, NP_], I32, tag="tokf")
        tokb_t = P.tile([128, NP_], I32, tag="tokb")
        wkf_t = P.tile([EA, G4], BF16, tag="wkf")
        wkb_t = P.tile([EA, G4], BF16, tag="wkb")
        wrf_t = P.tile([U, G4], BF16, tag="wrf")
        wrb_t = P.tile([U, G4], BF16, tag="wrb")
        ckf_t = P.tile([U, K], BF16, tag="ckf")
        ckb_t = P.tile([U, K], BF16, tag="ckb")
        ae_t = P.tile([K, K], BF16, tag="ae")
        aet_t = P.tile([K, K], BF16, tag="aet")
        embias_t = P.tile([K, 1], F32, tag="embias")
        ident_t = P.tile([128, 128], F32, tag="ident")
        xTf = P.tile([EA, NP_ * CB], BF16, tag="xTf")
        xTb = P.tile([EA, NP_ * CB], BF16, tag="xTb")
        h_f = P.tile([U, NP_ * CB], BF16, tag="hf")
        h_b = P.tile([U, (NP_ + WARM) * CB], BF16, tag="hb")
        em_e = P.tile([K, T * BL], BF16, tag="eme")
        ones_t = P.tile([K, 1], F32, tag="ones")
        neg1_t = P.tile([128, 1], F32, tag="neg1")

        nc.sync.dma_start(tokf_t[:], tok_f[:])
        nc.sync.dma_start(tokb_t[:], tok_b[:])
        nc.sync.dma_start(wkf_t[:], wk_f[:])
        nc.sync.dma_start(wkb_t[:], wk_b[:])
        nc.sync.dma_start(wrf_t[:], wr_f[:])
        nc.sync.dma_start(wrb_t[:], wr_b[:])
        nc.sync.dma_start(ckf_t[:], ck_f[:])
        nc.sync.dma_start(ckb_t[:], ck_b[:])
        nc.sync.dma_start(ae_t[:], ae[:])
        nc.sync.dma_start(aet_t[:], aet[:])
        nc.sync.dma_start(embias_t[:], embias[:])
        nc.sync.dma_start(ident_t[:], ident[:])
        nc.vector.memset(ones_t[:], 1.0)
        nc.vector.memset(neg1_t[:], -1.0)

        wk_ts = (wkf_t, wkb_t)
        wr_ts = (wrf_t, wrb_t)
        tok_ts = (tokf_t, tokb_t)
        xT_ts = (xTf, xTb)
        h_ts = (h_f, h_b)

        with ExitStack() as sctx:
            gat = sctx.enter_context(tc.tile_pool(name="gat", bufs=4))
            tp_ps = sctx.enter_context(tc.tile_pool(name="tp_ps", bufs=2, space="PSUM"))
            zpool = tuple(
                sctx.enter_context(tc.tile_pool(name=f"z{i}", bufs=1, space="PSUM"))
                for i in range(2))
            sgpool = tuple(
                sctx.enter_context(tc.tile_pool(name=f"sg{i}", bufs=3))
                for i in range(2))
            scrpool = tuple(
                sctx.enter_context(tc.tile_pool(name=f"scr{i}", bufs=2))
                for i in range(2))
            thpool = tuple(
                sctx.enter_context(tc.tile_pool(name=f"th{i}", bufs=2))
                for i in range(2))

            def emit_block(d, s):
                g = gat.tile([128, EA], F32, tag="g", name="g")
                nc.gpsimd.indirect_dma_start(
                    out=g[:],
                    out_offset=None,
                    in_=emb[:],
                    in_offset=bass.IndirectOffsetOnAxis(
                        ap=tok_ts[d][:, s:s + 1], axis=0),
                )
                pt = tp_ps.tile([EA, 128], F32, tag="pt", name="pt")
                nc.tensor.transpose(pt[:], g[:], ident_t[:])
                nc.vector.tensor_copy(xT_ts[d][:, s * CB:(s + 1) * CB], pt[:])

            # h block position: fwd writes block s; bwd writes block
            # (NP_ + WARM - 1) - s so that real blocks [WARM, NP_) of h_f and
            # h_b are time-aligned (bwd chunk slots are host-relabeled).
            def hpos(d, s):
                return s if d == 0 else (NP_ + WARM - 1) - s

            gi_next = [0, 0]
            for s in range(4):
                emit_block(0, s)
                emit_block(1, s)
            gfetched = 4

            sg_cur = [None, None]
            z_cur = [None, None]
            th = [None, None]
            for s in range(NP_):
                while gfetched < min(NP_, s + 4):
                    emit_block(0, gfetched)
                    emit_block(1, gfetched)
                    gfetched += 1
                # chunk-0 boundary reset: before the s=WARM h-MMs, zero the
                # exact-start chunk's h and set its cell state to zero
                # (chat = 1/2).  fwd exact chunk is slot 0; bwd is slot NCH-1.
                if s == WARM:
                    nc.vector.memset(
                        h_f[:, (WARM - 1) * CB:(WARM - 1) * CB + BL], 0.0)
                    qb = (NCH - 1) * BL
                    nc.vector.memset(
                        h_b[:, hpos(1, WARM - 1) * CB + qb:
                            hpos(1, WARM - 1) * CB + qb + BL], 0.0)
                    nc.vector.memset(sg_cur[0][:, 4 * CB:4 * CB + BL], 0.5)
                    nc.vector.memset(sg_cur[1][:, 4 * CB + qb:5 * CB], 0.5)
                # PE: x-MMs then h-MMs per chain.
                for d in (0, 1):
                    z_cur[d] = zpool[d].tile([128, 4 * CB], F32, tag="z",
                                             name=f"z{d}")
                    xs = xT_ts[d][:, s * CB:(s + 1) * CB]
                    for gi in range(4):
                        nc.tensor.matmul(
                            z_cur[d][:, gi * CB:(gi + 1) * CB],
                            wk_ts[d][:, gi * U:(gi + 1) * U],
                            xs,
                            start=(gi == 0),
                            stop=(s == 0 and gi == 3),
                        )
                    if s > 0:
                        hs = h_ts[d][:, hpos(d, s - 1) * CB:
                                     (hpos(d, s - 1) + 1) * CB]
                        for gi in range(4):
                            nc.tensor.matmul(
                                z_cur[d][:, gi * CB:(gi + 1) * CB],
                                wr_ts[d][:, gi * U:(gi + 1) * U],
                                hs,
                                start=False,
                                stop=(gi == 3),
                            )
                # ACT: sigmoids.
                for d in (0, 1):
                    if s == 0:
                        sg_cur[d] = sgpool[d].tile([128, 5 * CB], BF16,
                                                   tag="sg", name=f"sg{d}")
                    nc.scalar.activation(sg_cur[d][:, 0:4 * CB], z_cur[d][:],
                                         AF.Sigmoid)
                # DVE: cell update (chat = c/2 + 1/2 storage).
                sg_next = [None, None]
                for d in (0, 1):
                    sg_next[d] = sgpool[d].tile([128, 5 * CB], BF16, tag="sg",
                                                name=f"sg{d}")
                    sg = sg_cur[d]
                    cdst = sg_next[d][:, 4 * CB:5 * CB]
                    if s == 0:
                        a0 = scrpool[d].tile([128, CB], BF16, tag="ab",
                                             name=f"ab{d}")
                        nc.vector.scalar_tensor_tensor(
                            a0[:], sg[:, 3 * CB:4 * CB], 0.5, sg[:, 0:CB],
                            ALU.subtract, ALU.mult)
                        nc.vector.tensor_scalar(cdst, a0[:], 0.5, None, ALU.add)
                    else:
                        ab = scrpool[d].tile([128, 2 * CB], BF16, tag="ab",
                                             name=f"ab{d}")
                        nc.vector.scalar_tensor_tensor(
                            ab[:], sg[:, 3 * CB:5 * CB], 0.5, sg[:, 0:2 * CB],
                            ALU.subtract, ALU.mult)
                        nc.vector.scalar_tensor_tensor(
                            cdst, ab[:, 0:CB], 0.5, ab[:, CB:2 * CB],
                            ALU.add, ALU.add)
                # ACT: tanh(c) = tanh(2*chat - 1).
                for d in (0, 1):
                    th[d] = thpool[d].tile([128, CB], BF16, tag="th",
                                           name=f"th{d}")
                    nc.scalar.activation(th[d][:], sg_next[d][:, 4 * CB:5 * CB],
                                         AF.Tanh, bias=neg1_t[:], scale=2.0)
                # DVE: h = so * tanh(c).
                for d in (0, 1):
                    nc.vector.tensor_tensor(
                        h_ts[d][:, hpos(d, s) * CB:(hpos(d, s) + 1) * CB],
                        sg_cur[d][:, 2 * CB:3 * CB], th[d][:], ALU.mult)
                    sg_cur[d] = sg_next[d]

        # keep the exp/ln table phase strictly after the sigmoid/tanh phase
        tc.no_sync_barrier()

        EMC = 512
        with (
            tc.tile_pool(name="emps", bufs=4, space="PSUM") as emps,
            tc.tile_pool(name="crf", bufs=4) as crf,
            tc.tile_pool(name="crfps", bufs=2, space="PSUM") as crfps,
        ):
            nchunk = T * BL // EMC
            emorder = []
            lo, hi = 0, nchunk - 1
            while lo <= hi:
                emorder.append(lo)
                if hi != lo:
                    emorder.append(hi)
                lo += 1
                hi -= 1
            RB = WARM * CB  # start of the real (non-warmup) region
            for ch in emorder:
                ep = emps.tile([K, EMC], F32, tag="ep")
                nc.tensor.matmul(ep[:], ckf_t[:],
                                 h_f[:, RB + ch * EMC:RB + (ch + 1) * EMC],
                                 start=True, stop=False)
                nc.tensor.matmul(ep[:], ckb_t[:],
                                 h_b[:, RB + ch * EMC:RB + (ch + 1) * EMC],
                                 start=False, stop=True)
                nc.scalar.activation(em_e[:, ch * EMC:(ch + 1) * EMC], ep[:],
                                     AF.Exp, bias=embias_t[:], scale=1.0)

            def ecol(tau):
                return (tau % (T // NCH)) * CB + (tau // (T // NCH)) * BL

            a_cur = crf.tile([K, BL], BF16, tag="a")
            nc.vector.tensor_copy(a_cur[:], em_e[:, ecol(0):ecol(0) + BL])
            b_cur = crf.tile([K, BL], BF16, tag="b")
            nc.vector.tensor_copy(b_cur[:], em_e[:, ecol(T - 1):ecol(T - 1) + BL])

            for s in range(1, MID + 1):
                aps = crfps.tile([K, BL], F32, tag="aps")
                nc.tensor.matmul(aps[:], ae_t[:], a_cur[:], start=True, stop=True)
                a_new = crf.tile([K, BL], BF16, tag="a")
                nc.vector.tensor_tensor(a_new[:], aps[:],
                                        em_e[:, ecol(s):ecol(s) + BL], ALU.mult)
                a_cur = a_new

                if s <= MID - 1:
                    t_b = T - 1 - s
                    bps = crfps.tile([K, BL], F32, tag="bps")
                    nc.tensor.matmul(bps[:], aet_t[:], b_cur[:], start=True, stop=True)
                    b_new = crf.tile([K, BL], BF16, tag="b")
                    if t_b == MID:
                        nc.vector.tensor_copy(b_new[:], bps[:])
                    else:
                        nc.vector.tensor_tensor(b_new[:], bps[:],
                                                em_e[:, ecol(t_b):ecol(t_b) + BL],
                                                ALU.mult)
                    b_cur = b_new

            prod = crf.tile([K, BL], F32, tag="prod")
            nc.vector.tensor_tensor(prod[:], a_cur[:], b_cur[:], ALU.mult)
            sps = crfps.tile([1, BL], F32, tag="aps")
            nc.tensor.matmul(sps[:], ones_t[:], prod[:], start=True, stop=True)
            logz = crf.tile([1, BL], F32, tag="logz")
            nc.scalar.activation(logz[:], sps[:], AF.Ln)
            logz2 = crf.tile([1, BL], F32, tag="logz2")
            nc.vector.tensor_scalar(logz2[:], logz[:], float(T * DELTA), None, ALU.add)
            nc.sync.dma_start(out[:], logz2[:])

    nc.compile()
    return nc


def _gate_permute(w):
    """Reorder gate blocks from reference (i,f,g,o) to kernel (i,f,o,g) and
    pre-double the g block so tanh(g) = 2*sigmoid(2g)-1 needs only sigmoid."""
    i, f, g, o = np.split(w, 4, axis=-1)
    return np.concatenate([i, f, o, 2.0 * g], axis=-1)


def _stage(tokens, emb, Wk_f, Wr_f, b_f, Wk_b, Wr_b, b_b, crf_kernel, crf_bias,
           trans):
    """Host staging: build the per-core input maps."""
    emb_aug = np.concatenate(
        [emb, np.ones((V, 1), np.float32), np.zeros((V, EA - E - 1), np.float32)], 1)
    wk_aug_f = np.concatenate([Wk_f, b_f[None], np.zeros((EA - E - 1, G4), np.float32)], 0)
    wk_aug_b = np.concatenate([Wk_b, b_b[None], np.zeros((EA - E - 1, G4), np.float32)], 0)
    Ae = np.exp(trans).astype(np.float32)

    shared = {
        "emb": emb_aug,
        "wk_f": np.ascontiguousarray(_gate_permute(wk_aug_f)).astype(NPBF16),
        "wk_b": np.ascontiguousarray(_gate_permute(wk_aug_b)).astype(NPBF16),
        "wr_f": np.ascontiguousarray(_gate_permute(Wr_f)).astype(NPBF16),
        "wr_b": np.ascontiguousarray(_gate_permute(Wr_b)).astype(NPBF16),
        "ck_f": np.ascontiguousarray(crf_kernel[:U]).astype(NPBF16),
        "ck_b": np.ascontiguousarray(crf_kernel[U:]).astype(NPBF16),
        "ae": np.ascontiguousarray(Ae).astype(NPBF16),
        "aet": np.ascontiguousarray(Ae.T).astype(NPBF16),
        "embias": (crf_bias - DELTA).astype(np.float32).reshape(K, 1),
        "ident": np.eye(128, dtype=np.float32),
    }

    NCH, WARM = 8, 8
    NP_ = T // NCH + WARM
    CL = T // NCH
    ss = np.arange(NP_)[:, None]
    jj = np.arange(NCH)[None, :]
    tf = np.clip(CL * jj - WARM + ss, 0, T - 1)           # [NP_, NCH] fwd times
    tb = np.clip(CL - 1 + WARM + CL * jj - ss, 0, T - 1)  # bwd (slot-relabeled)
    in_maps = []
    for c in range(NCORES):
        tc_ = tokens[c * BL:(c + 1) * BL].astype(np.int32)  # [16, T]
        tok_f = tc_[:, tf].transpose(2, 0, 1).reshape(NCH * BL, NP_)
        tok_b = tc_[:, tb].transpose(2, 0, 1).reshape(NCH * BL, NP_)
        in_maps.append({"tok_f": np.ascontiguousarray(tok_f),
                        "tok_b": np.ascontiguousarray(tok_b), **shared})
    return in_maps


_PROGRAM_CACHE = {}


def kernel(tokens, emb, Wk_f, Wr_f, b_f, Wk_b, Wr_b, b_b, crf_kernel, crf_bias, trans):
    from concourse.bass_utils import run_bass_kernel_spmd

    tokens = np.asarray(tokens)
    emb = np.asarray(emb, dtype=np.float32)
    Wk_f = np.asarray(Wk_f, np.float32); Wr_f = np.asarray(Wr_f, np.float32)
    Wk_b = np.asarray(Wk_b, np.float32); Wr_b = np.asarray(Wr_b, np.float32)
    b_f = np.asarray(b_f, np.float32); b_b = np.asarray(b_b, np.float32)
    crf_kernel = np.asarray(crf_kernel, np.float32)
    crf_bias = np.asarray(crf_bias, np.float32)
    trans = np.asarray(trans, np.float32)

    if "nc" not in _PROGRAM_CACHE:
        _PROGRAM_CACHE["nc"] = _build_program()
    nc = _PROGRAM_CACHE["nc"]

    in_maps = _stage(tokens, emb, Wk_f, Wr_f, b_f, Wk_b, Wr_b, b_b,
                     crf_kernel, crf_bias, trans)
    res = run_bass_kernel_spmd(nc, in_maps, core_ids=list(range(NCORES)))
    outs = [res.results[c]["out"].reshape(BL).astype(np.float32) for c in range(NCORES)]
    return np.concatenate(outs, axis=0)


# revision 21
# speedup vs baseline: 1.4787x; 1.0514x over previous
"""Trainium2 Bass kernel for nn_LstmCrf: bidirectional LSTM + CRF log-partition.

Contract: kernel(**inputs) takes the FULL unsharded inputs and returns the FULL
output logZ [128] f32. Internally shards the batch (128 rows) across 8
NeuronCores (16 rows each), runs one SPMD Bass/Tile program, and concatenates
the per-core results.

Problem shapes (hardcoded): B=128, T=512, V=50000, E=100, U=128, K=32.

v2 design (vs lockstep v1 @2.21us/step): the fwd and bwd LSTM scans run as two
DECOUPLED dependency chains, interleaved so each engine alternates between the
chains and the ~1.6us per-step chain latency of one chain hides behind the
other.  Per chain-step: 4 x-proj MMs are emitted one step ahead (fill PE idle),
4 h-proj MMs -> sigmoid ACT [128,64] -> 3 fused DVE ops for the cell update
(layout trick: sg tile [128,80] = [i f o g | c_prev] makes (si|sf)*(sgg|c_prev)
a single tensor_tensor) -> tanh ACT [128,16] -> 1 DVE h-mult straight into
h_all.  ACT queue order per step is [sig_f, sig_b, tanh_f, tanh_b].

Emissions: em_e = exp(em + bias - delta) bf16 as before.  CRF: exp-domain
meet-in-the-middle DP with BF16 transition matrices (fp32 lhsT costs 2 HW
matmuls per logical matmul on the PE; bf16 costs 1).
"""
import sys
from contextlib import ExitStack

import numpy as np

for p in ("/opt/trn_rl_repo", "/root/.axon_site/_ro/trn_rl_repo"):
    if p not in sys.path:
        sys.path.append(p)

import ml_dtypes

NPBF16 = ml_dtypes.bfloat16

B, T = 128, 512
V, E, U, K = 50000, 100, 128, 32
NCORES = 8
BL = B // NCORES          # 16 rows per core
EA = 104                  # padded embedding dim
G4 = 4 * U
DELTA = float(np.log(K))


def _build_program(T=T):
    import concourse.bacc as bacc
    import concourse.bass as bass
    import concourse.mybir as mybir
    import concourse.tile as tile

    F32 = mybir.dt.float32
    BF16 = mybir.dt.bfloat16
    I32 = mybir.dt.int32
    AF = mybir.ActivationFunctionType
    ALU = mybir.AluOpType

    NCH = 16                  # chunks per direction
    WARM = 8                  # warmup steps per chunk
    NP_ = T // NCH + WARM     # 80 lockstep periods
    CB = NCH * BL             # 128 cols per period block
    MID = T // 2

    nc = bacc.Bacc(None, target_bir_lowering=False, debug=False)

    tok_f = nc.dram_tensor("tok_f", [128, 2 * NP_], I32, kind="ExternalInput")
    tok_b = nc.dram_tensor("tok_b", [128, 2 * NP_], I32, kind="ExternalInput")
    emb = nc.dram_tensor("emb", [V, EA], F32, kind="ExternalInput")
    wk_f = nc.dram_tensor("wk_f", [EA, G4], BF16, kind="ExternalInput")
    wk_b = nc.dram_tensor("wk_b", [EA, G4], BF16, kind="ExternalInput")
    wr_f = nc.dram_tensor("wr_f", [U, G4], BF16, kind="ExternalInput")
    wr_b = nc.dram_tensor("wr_b", [U, G4], BF16, kind="ExternalInput")
    ck_f = nc.dram_tensor("ck_f", [U, K], BF16, kind="ExternalInput")
    ck_b = nc.dram_tensor("ck_b", [U, K], BF16, kind="ExternalInput")
    ae = nc.dram_tensor("ae", [K, K], BF16, kind="ExternalInput")
    aet = nc.dram_tensor("aet", [K, K], BF16, kind="ExternalInput")
    embias = nc.dram_tensor("embias", [K, 1], F32, kind="ExternalInput")
    ident = nc.dram_tensor("ident", [128, 128], F32, kind="ExternalInput")
    out = nc.dram_tensor("out", [1, BL], F32, kind="ExternalOutput")

    def block_order(nblk):
        order = []
        lo, hi = 0, nblk - 1
        while lo <= hi:
            order.append(lo)
            if hi != lo:
                order.append(hi)
            lo += 1
            hi -= 1
        return order

    with tile.TileContext(nc) as tc, ExitStack() as ctx:
        P = ctx.enter_context(tc.tile_pool(name="persist", bufs=1))
        tokf_t = P.tile([128, 2 * NP_], I32, tag="tokf")
        tokb_t = P.tile([128, 2 * NP_], I32, tag="tokb")
        wkf_t = P.tile([EA, G4], BF16, tag="wkf")
        wkb_t = P.tile([EA, G4], BF16, tag="wkb")
        wrf_t = P.tile([U, G4], BF16, tag="wrf")
        wrb_t = P.tile([U, G4], BF16, tag="wrb")
        ckf_t = P.tile([U, K], BF16, tag="ckf")
        ckb_t = P.tile([U, K], BF16, tag="ckb")
        ae_t = P.tile([K, K], BF16, tag="ae")
        aet_t = P.tile([K, K], BF16, tag="aet")
        embias_t = P.tile([K, 1], F32, tag="embias")
        ident_t = P.tile([128, 128], F32, tag="ident")
        xTf = P.tile([EA, NP_ * CB], BF16, tag="xTf")
        xTb = P.tile([EA, NP_ * CB], BF16, tag="xTb")
        h_f = P.tile([U, NP_ * CB], BF16, tag="hf")
        h_b = P.tile([U, (NP_ + WARM) * CB], BF16, tag="hb")
        em_e = P.tile([K, T * BL], BF16, tag="eme")
        ones_t = P.tile([K, 1], F32, tag="ones")
        neg1_t = P.tile([128, 1], F32, tag="neg1")

        nc.sync.dma_start(tokf_t[:], tok_f[:])
        nc.sync.dma_start(tokb_t[:], tok_b[:])
        nc.sync.dma_start(wkf_t[:], wk_f[:])
        nc.sync.dma_start(wkb_t[:], wk_b[:])
        nc.sync.dma_start(wrf_t[:], wr_f[:])
        nc.sync.dma_start(wrb_t[:], wr_b[:])
        nc.sync.dma_start(ckf_t[:], ck_f[:])
        nc.sync.dma_start(ckb_t[:], ck_b[:])
        nc.sync.dma_start(ae_t[:], ae[:])
        nc.sync.dma_start(aet_t[:], aet[:])
        nc.sync.dma_start(embias_t[:], embias[:])
        nc.sync.dma_start(ident_t[:], ident[:])
        nc.vector.memset(ones_t[:], 1.0)
        nc.vector.memset(neg1_t[:], -1.0)

        wk_ts = (wkf_t, wkb_t)
        wr_ts = (wrf_t, wrb_t)
        tok_ts = (tokf_t, tokb_t)
        xT_ts = (xTf, xTb)
        h_ts = (h_f, h_b)

        with ExitStack() as sctx:
            gat = sctx.enter_context(tc.tile_pool(name="gat", bufs=4))
            tp_ps = sctx.enter_context(tc.tile_pool(name="tp_ps", bufs=2, space="PSUM"))
            zpool = tuple(
                sctx.enter_context(tc.tile_pool(name=f"z{i}", bufs=1, space="PSUM"))
                for i in range(2))
            sgpool = tuple(
                sctx.enter_context(tc.tile_pool(name=f"sg{i}", bufs=3))
                for i in range(2))
            scrpool = tuple(
                sctx.enter_context(tc.tile_pool(name=f"scr{i}", bufs=2))
                for i in range(2))
            thpool = tuple(
                sctx.enter_context(tc.tile_pool(name=f"th{i}", bufs=2))
                for i in range(2))

            def emit_block(d, s):
                for half in (0, 1):
                    g = gat.tile([128, EA], F32, tag="g", name="g")
                    nc.gpsimd.indirect_dma_start(
                        out=g[:],
                        out_offset=None,
                        in_=emb[:],
                        in_offset=bass.IndirectOffsetOnAxis(
                            ap=tok_ts[d][:, 2 * s + half:2 * s + half + 1],
                            axis=0),
                    )
                    pt = tp_ps.tile([EA, 128], F32, tag="pt", name="pt")
                    nc.tensor.transpose(pt[:], g[:], ident_t[:])
                    nc.vector.tensor_copy(
                        xT_ts[d][:, s * CB + half * 128:
                                 s * CB + (half + 1) * 128], pt[:])

            # h block position: fwd writes block s; bwd writes block
            # (NP_ + WARM - 1) - s so that real blocks [WARM, NP_) of h_f and
            # h_b are time-aligned (bwd chunk slots are host-relabeled).
            def hpos(d, s):
                return s if d == 0 else (NP_ + WARM - 1) - s

            gi_next = [0, 0]
            for s in range(4):
                emit_block(0, s)
                emit_block(1, s)
            gfetched = 4

            sg_cur = [None, None]
            z_cur = [None, None]
            th = [None, None]
            for s in range(NP_):
                while gfetched < min(NP_, s + 4):
                    emit_block(0, gfetched)
                    emit_block(1, gfetched)
                    gfetched += 1
                # chunk-0 boundary reset: before the s=WARM h-MMs, zero the
                # exact-start chunk's h and set its cell state to zero
                # (chat = 1/2).  fwd exact chunk is slot 0; bwd is slot NCH-1.
                if s == WARM:
                    nc.vector.memset(
                        h_f[:, (WARM - 1) * CB:(WARM - 1) * CB + BL], 0.0)
                    qb = (NCH - 1) * BL
                    nc.vector.memset(
                        h_b[:, hpos(1, WARM - 1) * CB + qb:
                            hpos(1, WARM - 1) * CB + qb + BL], 0.0)
                    nc.vector.memset(sg_cur[0][:, 4 * CB:4 * CB + BL], 0.5)
                    nc.vector.memset(sg_cur[1][:, 4 * CB + qb:5 * CB], 0.5)
                # PE: x-MMs then h-MMs per chain.
                for d in (0, 1):
                    z_cur[d] = zpool[d].tile([128, 4 * CB], F32, tag="z",
                                             name=f"z{d}")
                    xs = xT_ts[d][:, s * CB:(s + 1) * CB]
                    for gi in range(4):
                        nc.tensor.matmul(
                            z_cur[d][:, gi * CB:(gi + 1) * CB],
                            wk_ts[d][:, gi * U:(gi + 1) * U],
                            xs,
                            start=(gi == 0 or gi == 2),
                            stop=(s == 0 and gi == 3),
                        )
                    if s > 0:
                        hs = h_ts[d][:, hpos(d, s - 1) * CB:
                                     (hpos(d, s - 1) + 1) * CB]
                        for gi in range(4):
                            nc.tensor.matmul(
                                z_cur[d][:, gi * CB:(gi + 1) * CB],
                                wr_ts[d][:, gi * U:(gi + 1) * U],
                                hs,
                                start=False,
                                stop=(gi == 3),
                            )
                # ACT: sigmoids.
                for d in (0, 1):
                    if s == 0:
                        sg_cur[d] = sgpool[d].tile([128, 5 * CB], BF16,
                                                   tag="sg", name=f"sg{d}")
                    nc.scalar.activation(sg_cur[d][:, 0:4 * CB], z_cur[d][:],
                                         AF.Sigmoid)
                # DVE: cell update (chat = c/2 + 1/2 storage).
                sg_next = [None, None]
                for d in (0, 1):
                    sg_next[d] = sgpool[d].tile([128, 5 * CB], BF16, tag="sg",
                                                name=f"sg{d}")
                    sg = sg_cur[d]
                    cdst = sg_next[d][:, 4 * CB:5 * CB]
                    if s == 0:
                        a0 = scrpool[d].tile([128, CB], BF16, tag="ab",
                                             name=f"ab{d}")
                        nc.vector.scalar_tensor_tensor(
                            a0[:], sg[:, 3 * CB:4 * CB], 0.5, sg[:, 0:CB],
                            ALU.subtract, ALU.mult)
                        nc.vector.tensor_scalar(cdst, a0[:], 0.5, None, ALU.add)
                    else:
                        ab = scrpool[d].tile([128, 2 * CB], BF16, tag="ab",
                                             name=f"ab{d}")
                        nc.vector.scalar_tensor_tensor(
                            ab[:], sg[:, 3 * CB:5 * CB], 0.5, sg[:, 0:2 * CB],
                            ALU.subtract, ALU.mult)
                        nc.vector.scalar_tensor_tensor(
                            cdst, ab[:, 0:CB], 0.5, ab[:, CB:2 * CB],
                            ALU.add, ALU.add)
                # ACT: tanh(c) = tanh(2*chat - 1).
                for d in (0, 1):
                    th[d] = thpool[d].tile([128, CB], BF16, tag="th",
                                           name=f"th{d}")
                    nc.scalar.activation(th[d][:], sg_next[d][:, 4 * CB:5 * CB],
                                         AF.Tanh, bias=neg1_t[:], scale=2.0)
                # DVE: h = so * tanh(c).
                for d in (0, 1):
                    nc.vector.tensor_tensor(
                        h_ts[d][:, hpos(d, s) * CB:(hpos(d, s) + 1) * CB],
                        sg_cur[d][:, 2 * CB:3 * CB], th[d][:], ALU.mult)
                    sg_cur[d] = sg_next[d]

        # keep the exp/ln table phase strictly after the sigmoid/tanh phase
        tc.no_sync_barrier()

        EMC = 512
        with (
            tc.tile_pool(name="emps", bufs=4, space="PSUM") as emps,
            tc.tile_pool(name="crf", bufs=4) as crf,
            tc.tile_pool(name="crfps", bufs=2, space="PSUM") as crfps,
        ):
            nchunk = T * BL // EMC
            emorder = []
            lo, hi = 0, nchunk - 1
            while lo <= hi:
                emorder.append(lo)
                if hi != lo:
                    emorder.append(hi)
                lo += 1
                hi -= 1
            RB = WARM * CB  # start of the real (non-warmup) region
            for ch in emorder:
                ep = emps.tile([K, EMC], F32, tag="ep")
                nc.tensor.matmul(ep[:], ckf_t[:],
                                 h_f[:, RB + ch * EMC:RB + (ch + 1) * EMC],
                                 start=True, stop=False)
                nc.tensor.matmul(ep[:], ckb_t[:],
                                 h_b[:, RB + ch * EMC:RB + (ch + 1) * EMC],
                                 start=False, stop=True)
                nc.scalar.activation(em_e[:, ch * EMC:(ch + 1) * EMC], ep[:],
                                     AF.Exp, bias=embias_t[:], scale=1.0)

            def ecol(tau):
                return (tau % (T // NCH)) * CB + (tau // (T // NCH)) * BL

            a_cur = crf.tile([K, BL], BF16, tag="a")
            nc.vector.tensor_copy(a_cur[:], em_e[:, ecol(0):ecol(0) + BL])
            b_cur = crf.tile([K, BL], BF16, tag="b")
            nc.vector.tensor_copy(b_cur[:], em_e[:, ecol(T - 1):ecol(T - 1) + BL])

            for s in range(1, MID + 1):
                aps = crfps.tile([K, BL], F32, tag="aps")
                nc.tensor.matmul(aps[:], ae_t[:], a_cur[:], start=True, stop=True)
                a_new = crf.tile([K, BL], BF16, tag="a")
                nc.vector.tensor_tensor(a_new[:], aps[:],
                                        em_e[:, ecol(s):ecol(s) + BL], ALU.mult)
                a_cur = a_new

                if s <= MID - 1:
                    t_b = T - 1 - s
                    bps = crfps.tile([K, BL], F32, tag="bps")
                    nc.tensor.matmul(bps[:], aet_t[:], b_cur[:], start=True, stop=True)
                    b_new = crf.tile([K, BL], BF16, tag="b")
                    if t_b == MID:
                        nc.vector.tensor_copy(b_new[:], bps[:])
                    else:
                        nc.vector.tensor_tensor(b_new[:], bps[:],
                                                em_e[:, ecol(t_b):ecol(t_b) + BL],
                                                ALU.mult)
                    b_cur = b_new

            prod = crf.tile([K, BL], F32, tag="prod")
            nc.vector.tensor_tensor(prod[:], a_cur[:], b_cur[:], ALU.mult)
            sps = crfps.tile([1, BL], F32, tag="aps")
            nc.tensor.matmul(sps[:], ones_t[:], prod[:], start=True, stop=True)
            logz = crf.tile([1, BL], F32, tag="logz")
            nc.scalar.activation(logz[:], sps[:], AF.Ln)
            logz2 = crf.tile([1, BL], F32, tag="logz2")
            nc.vector.tensor_scalar(logz2[:], logz[:], float(T * DELTA), None, ALU.add)
            nc.sync.dma_start(out[:], logz2[:])

    nc.compile()
    return nc


def _gate_permute(w):
    """Reorder gate blocks from reference (i,f,g,o) to kernel (i,f,o,g) and
    pre-double the g block so tanh(g) = 2*sigmoid(2g)-1 needs only sigmoid."""
    i, f, g, o = np.split(w, 4, axis=-1)
    return np.concatenate([i, f, o, 2.0 * g], axis=-1)


def _stage(tokens, emb, Wk_f, Wr_f, b_f, Wk_b, Wr_b, b_b, crf_kernel, crf_bias,
           trans):
    """Host staging: build the per-core input maps."""
    emb_aug = np.concatenate(
        [emb, np.ones((V, 1), np.float32), np.zeros((V, EA - E - 1), np.float32)], 1)
    wk_aug_f = np.concatenate([Wk_f, b_f[None], np.zeros((EA - E - 1, G4), np.float32)], 0)
    wk_aug_b = np.concatenate([Wk_b, b_b[None], np.zeros((EA - E - 1, G4), np.float32)], 0)
    Ae = np.exp(trans).astype(np.float32)

    shared = {
        "emb": emb_aug,
        "wk_f": np.ascontiguousarray(_gate_permute(wk_aug_f)).astype(NPBF16),
        "wk_b": np.ascontiguousarray(_gate_permute(wk_aug_b)).astype(NPBF16),
        "wr_f": np.ascontiguousarray(_gate_permute(Wr_f)).astype(NPBF16),
        "wr_b": np.ascontiguousarray(_gate_permute(Wr_b)).astype(NPBF16),
        "ck_f": np.ascontiguousarray(crf_kernel[:U]).astype(NPBF16),
        "ck_b": np.ascontiguousarray(crf_kernel[U:]).astype(NPBF16),
        "ae": np.ascontiguousarray(Ae).astype(NPBF16),
        "aet": np.ascontiguousarray(Ae.T).astype(NPBF16),
        "embias": (crf_bias - DELTA).astype(np.float32).reshape(K, 1),
        "ident": np.eye(128, dtype=np.float32),
    }

    NCH, WARM = 16, 8
    NP_ = T // NCH + WARM
    CL = T // NCH
    ss = np.arange(NP_)[:, None]
    jj = np.arange(NCH)[None, :]
    tf = np.clip(CL * jj - WARM + ss, 0, T - 1)           # [NP_, NCH] fwd times
    tb = np.clip(CL - 1 + WARM + CL * jj - ss, 0, T - 1)  # bwd (slot-relabeled)

    def tokmat(tc_, tm):
        full = tc_[:, tm].transpose(2, 0, 1).reshape(NCH * BL, NP_)  # [256, NP_]
        tk = np.empty((128, 2 * NP_), np.int32)
        tk[:, 0::2] = full[0:128]
        tk[:, 1::2] = full[128:256]
        return np.ascontiguousarray(tk)

    in_maps = []
    for c in range(NCORES):
        tc_ = tokens[c * BL:(c + 1) * BL].astype(np.int32)  # [16, T]
        in_maps.append({"tok_f": tokmat(tc_, tf),
                        "tok_b": tokmat(tc_, tb), **shared})
    return in_maps


_PROGRAM_CACHE = {}


def kernel(tokens, emb, Wk_f, Wr_f, b_f, Wk_b, Wr_b, b_b, crf_kernel, crf_bias, trans):
    from concourse.bass_utils import run_bass_kernel_spmd

    tokens = np.asarray(tokens)
    emb = np.asarray(emb, dtype=np.float32)
    Wk_f = np.asarray(Wk_f, np.float32); Wr_f = np.asarray(Wr_f, np.float32)
    Wk_b = np.asarray(Wk_b, np.float32); Wr_b = np.asarray(Wr_b, np.float32)
    b_f = np.asarray(b_f, np.float32); b_b = np.asarray(b_b, np.float32)
    crf_kernel = np.asarray(crf_kernel, np.float32)
    crf_bias = np.asarray(crf_bias, np.float32)
    trans = np.asarray(trans, np.float32)

    if "nc" not in _PROGRAM_CACHE:
        _PROGRAM_CACHE["nc"] = _build_program()
    nc = _PROGRAM_CACHE["nc"]

    in_maps = _stage(tokens, emb, Wk_f, Wr_f, b_f, Wk_b, Wr_b, b_b,
                     crf_kernel, crf_bias, trans)
    res = run_bass_kernel_spmd(nc, in_maps, core_ids=list(range(NCORES)))
    outs = [res.results[c]["out"].reshape(BL).astype(np.float32) for c in range(NCORES)]
    return np.concatenate(outs, axis=0)


# revision 23
# speedup vs baseline: 1.6554x; 1.1195x over previous
"""Trainium2 Bass kernel for nn_LstmCrf: bidirectional LSTM + CRF log-partition.

Contract: kernel(**inputs) takes the FULL unsharded inputs and returns the FULL
output logZ [128] f32. Internally shards the batch (128 rows) across 8
NeuronCores (16 rows each), runs one SPMD Bass/Tile program, and concatenates
the per-core results.

Problem shapes (hardcoded): B=128, T=512, V=50000, E=100, U=128, K=32.

v2 design (vs lockstep v1 @2.21us/step): the fwd and bwd LSTM scans run as two
DECOUPLED dependency chains, interleaved so each engine alternates between the
chains and the ~1.6us per-step chain latency of one chain hides behind the
other.  Per chain-step: 4 x-proj MMs are emitted one step ahead (fill PE idle),
4 h-proj MMs -> sigmoid ACT [128,64] -> 3 fused DVE ops for the cell update
(layout trick: sg tile [128,80] = [i f o g | c_prev] makes (si|sf)*(sgg|c_prev)
a single tensor_tensor) -> tanh ACT [128,16] -> 1 DVE h-mult straight into
h_all.  ACT queue order per step is [sig_f, sig_b, tanh_f, tanh_b].

Emissions: em_e = exp(em + bias - delta) bf16 as before.  CRF: exp-domain
meet-in-the-middle DP with BF16 transition matrices (fp32 lhsT costs 2 HW
matmuls per logical matmul on the PE; bf16 costs 1).
"""
import sys
from contextlib import ExitStack

import numpy as np

for p in ("/opt/trn_rl_repo", "/root/.axon_site/_ro/trn_rl_repo"):
    if p not in sys.path:
        sys.path.append(p)

import ml_dtypes

NPBF16 = ml_dtypes.bfloat16

B, T = 128, 512
V, E, U, K = 50000, 100, 128, 32
NCORES = 8
BL = B // NCORES          # 16 rows per core
EA = 104                  # padded embedding dim
G4 = 4 * U
DELTA = float(np.log(K))


def _build_program(T=T):
    import concourse.bacc as bacc
    import concourse.bass as bass
    import concourse.mybir as mybir
    import concourse.tile as tile

    F32 = mybir.dt.float32
    BF16 = mybir.dt.bfloat16
    I32 = mybir.dt.int32
    AF = mybir.ActivationFunctionType
    ALU = mybir.AluOpType

    NCH = 16                  # chunks per direction
    WARM = 8                  # warmup steps per chunk
    NP_ = T // NCH + WARM     # 80 lockstep periods
    CB = NCH * BL             # 128 cols per period block
    MID = T // 2

    nc = bacc.Bacc(None, target_bir_lowering=False, debug=False)

    tok_f = nc.dram_tensor("tok_f", [128, 2 * NP_], I32, kind="ExternalInput")
    tok_b = nc.dram_tensor("tok_b", [128, 2 * NP_], I32, kind="ExternalInput")
    emb = nc.dram_tensor("emb", [V, EA], F32, kind="ExternalInput")
    wk_f = nc.dram_tensor("wk_f", [EA, G4], BF16, kind="ExternalInput")
    wk_b = nc.dram_tensor("wk_b", [EA, G4], BF16, kind="ExternalInput")
    wr_f = nc.dram_tensor("wr_f", [U, G4], BF16, kind="ExternalInput")
    wr_b = nc.dram_tensor("wr_b", [U, G4], BF16, kind="ExternalInput")
    ck_f = nc.dram_tensor("ck_f", [U, K], BF16, kind="ExternalInput")
    ck_b = nc.dram_tensor("ck_b", [U, K], BF16, kind="ExternalInput")
    ae = nc.dram_tensor("ae", [K, K], BF16, kind="ExternalInput")
    aet = nc.dram_tensor("aet", [K, K], BF16, kind="ExternalInput")
    embias = nc.dram_tensor("embias", [K, 1], F32, kind="ExternalInput")
    ident = nc.dram_tensor("ident", [128, 128], F32, kind="ExternalInput")
    out = nc.dram_tensor("out", [1, BL], F32, kind="ExternalOutput")

    def block_order(nblk):
        order = []
        lo, hi = 0, nblk - 1
        while lo <= hi:
            order.append(lo)
            if hi != lo:
                order.append(hi)
            lo += 1
            hi -= 1
        return order

    with tile.TileContext(nc) as tc, ExitStack() as ctx:
        P = ctx.enter_context(tc.tile_pool(name="persist", bufs=1))
        tokf_t = P.tile([128, 2 * NP_], I32, tag="tokf")
        tokb_t = P.tile([128, 2 * NP_], I32, tag="tokb")
        wkf_t = P.tile([EA, G4], BF16, tag="wkf")
        wkb_t = P.tile([EA, G4], BF16, tag="wkb")
        wrf_t = P.tile([U, G4], BF16, tag="wrf")
        wrb_t = P.tile([U, G4], BF16, tag="wrb")
        ckf_t = P.tile([U, K], BF16, tag="ckf")
        ckb_t = P.tile([U, K], BF16, tag="ckb")
        ae_t = P.tile([K, K], BF16, tag="ae")
        aet_t = P.tile([K, K], BF16, tag="aet")
        embias_t = P.tile([K, 1], F32, tag="embias")
        ident_t = P.tile([128, 128], F32, tag="ident")
        xTf = P.tile([EA, NP_ * CB], BF16, tag="xTf")
        xTb = P.tile([EA, WARM * CB], BF16, tag="xTb")
        h_f = P.tile([U, NP_ * CB], BF16, tag="hf")
        h_b = P.tile([U, (NP_ + WARM) * CB], BF16, tag="hb")
        em_e = P.tile([K, T * BL], BF16, tag="eme")
        ones_t = P.tile([K, 1], F32, tag="ones")
        neg1_t = P.tile([128, 1], F32, tag="neg1")

        nc.sync.dma_start(tokf_t[:], tok_f[:])
        nc.sync.dma_start(tokb_t[:], tok_b[:])
        nc.sync.dma_start(wkf_t[:], wk_f[:])
        nc.sync.dma_start(wkb_t[:], wk_b[:])
        nc.sync.dma_start(wrf_t[:], wr_f[:])
        nc.sync.dma_start(wrb_t[:], wr_b[:])
        nc.sync.dma_start(ckf_t[:], ck_f[:])
        nc.sync.dma_start(ckb_t[:], ck_b[:])
        nc.sync.dma_start(ae_t[:], ae[:])
        nc.sync.dma_start(aet_t[:], aet[:])
        nc.sync.dma_start(embias_t[:], embias[:])
        nc.sync.dma_start(ident_t[:], ident[:])
        nc.vector.memset(ones_t[:], 1.0)
        nc.vector.memset(neg1_t[:], -1.0)

        wk_ts = (wkf_t, wkb_t)
        wr_ts = (wrf_t, wrb_t)
        tok_ts = (tokf_t, tokb_t)
        xT_ts = (xTf, xTb)
        h_ts = (h_f, h_b)

        with ExitStack() as sctx:
            gat = sctx.enter_context(tc.tile_pool(name="gat", bufs=4))
            tp_ps = sctx.enter_context(tc.tile_pool(name="tp_ps", bufs=2, space="PSUM"))
            zpool = tuple(
                sctx.enter_context(tc.tile_pool(name=f"z{i}", bufs=1, space="PSUM"))
                for i in range(2))
            sgpool = tuple(
                sctx.enter_context(tc.tile_pool(name=f"sg{i}", bufs=3))
                for i in range(2))
            scrpool = tuple(
                sctx.enter_context(tc.tile_pool(name=f"scr{i}", bufs=2))
                for i in range(2))
            thpool = tuple(
                sctx.enter_context(tc.tile_pool(name=f"th{i}", bufs=2))
                for i in range(2))

            def emit_block(d, s):
                for half in (0, 1):
                    g = gat.tile([128, EA], F32, tag="g", name="g")
                    nc.gpsimd.indirect_dma_start(
                        out=g[:],
                        out_offset=None,
                        in_=emb[:],
                        in_offset=bass.IndirectOffsetOnAxis(
                            ap=tok_ts[d][:, 2 * s + half:2 * s + half + 1],
                            axis=0),
                    )
                    pt = tp_ps.tile([EA, 128], F32, tag="pt", name="pt")
                    nc.tensor.transpose(pt[:], g[:], ident_t[:])
                    nc.vector.tensor_copy(
                        xT_ts[d][:, s * CB + half * 128:
                                 s * CB + (half + 1) * 128], pt[:])

            # h block position: fwd writes block s; bwd writes block
            # (NP_ + WARM - 1) - s so that real blocks [WARM, NP_) of h_f and
            # h_b are time-aligned (bwd chunk slots are host-relabeled).
            def hpos(d, s):
                return s if d == 0 else (NP_ + WARM - 1) - s

            fetched = set()

            def fetch_for(p):
                # fwd low end (fwd chain consumes ascending), fwd high end
                # (bwd real phase consumes descending via the hpos mirror),
                # and bwd warmup blocks.
                for d, blk in ((0, p), (0, NP_ + WARM - 1 - p), (1, p)):
                    if d == 1 and blk >= WARM:
                        continue
                    if 0 <= blk < NP_ and (d, blk) not in fetched:
                        fetched.add((d, blk))
                        emit_block(d, blk)

            for p in range(4):
                fetch_for(p)

            sg_cur = [None, None]
            z_cur = [None, None]
            th = [None, None]
            for s in range(NP_):
                if s + 4 < NP_:
                    fetch_for(s + 4)
                # chunk-0 boundary reset: before the s=WARM h-MMs, zero the
                # exact-start chunk's h and set its cell state to zero
                # (chat = 1/2).  fwd exact chunk is slot 0; bwd is slot NCH-1.
                if s == WARM:
                    nc.vector.memset(
                        h_f[:, (WARM - 1) * CB:(WARM - 1) * CB + BL], 0.0)
                    qb = (NCH - 1) * BL
                    nc.vector.memset(
                        h_b[:, hpos(1, WARM - 1) * CB + qb:
                            hpos(1, WARM - 1) * CB + qb + BL], 0.0)
                    nc.vector.memset(sg_cur[0][:, 4 * CB:4 * CB + BL], 0.5)
                    nc.vector.memset(sg_cur[1][:, 4 * CB + qb:5 * CB], 0.5)
                # PE: x-MMs then h-MMs per chain.
                for d in (0, 1):
                    z_cur[d] = zpool[d].tile([128, 4 * CB], F32, tag="z",
                                             name=f"z{d}")
                    if d == 1 and s >= WARM:
                        xblk = NP_ + WARM - 1 - s
                        xs = xTf[:, xblk * CB:(xblk + 1) * CB]
                    else:
                        xs = xT_ts[d][:, s * CB:(s + 1) * CB]
                    for gi in range(4):
                        nc.tensor.matmul(
                            z_cur[d][:, gi * CB:(gi + 1) * CB],
                            wk_ts[d][:, gi * U:(gi + 1) * U],
                            xs,
                            start=(gi == 0 or gi == 2),
                            stop=(s == 0 and gi == 3),
                        )
                    if s > 0:
                        hs = h_ts[d][:, hpos(d, s - 1) * CB:
                                     (hpos(d, s - 1) + 1) * CB]
                        for gi in range(4):
                            nc.tensor.matmul(
                                z_cur[d][:, gi * CB:(gi + 1) * CB],
                                wr_ts[d][:, gi * U:(gi + 1) * U],
                                hs,
                                start=False,
                                stop=(gi == 3),
                            )
                # ACT: sigmoids.
                for d in (0, 1):
                    if s == 0:
                        sg_cur[d] = sgpool[d].tile([128, 5 * CB], BF16,
                                                   tag="sg", name=f"sg{d}")
                    nc.scalar.activation(sg_cur[d][:, 0:4 * CB], z_cur[d][:],
                                         AF.Sigmoid)
                # DVE: cell update (chat = c/2 + 1/2 storage).
                sg_next = [None, None]
                for d in (0, 1):
                    sg_next[d] = sgpool[d].tile([128, 5 * CB], BF16, tag="sg",
                                                name=f"sg{d}")
                    sg = sg_cur[d]
                    cdst = sg_next[d][:, 4 * CB:5 * CB]
                    if s == 0:
                        a0 = scrpool[d].tile([128, CB], BF16, tag="ab",
                                             name=f"ab{d}")
                        nc.vector.scalar_tensor_tensor(
                            a0[:], sg[:, 3 * CB:4 * CB], 0.5, sg[:, 0:CB],
                            ALU.subtract, ALU.mult)
                        nc.vector.tensor_scalar(cdst, a0[:], 0.5, None, ALU.add)
                    else:
                        ab = scrpool[d].tile([128, 2 * CB], BF16, tag="ab",
                                             name=f"ab{d}")
                        nc.vector.scalar_tensor_tensor(
                            ab[:], sg[:, 3 * CB:5 * CB], 0.5, sg[:, 0:2 * CB],
                            ALU.subtract, ALU.mult)
                        nc.vector.scalar_tensor_tensor(
                            cdst, ab[:, 0:CB], 0.5, ab[:, CB:2 * CB],
                            ALU.add, ALU.add)
                # ACT: tanh(c) = tanh(2*chat - 1).
                for d in (0, 1):
                    th[d] = thpool[d].tile([128, CB], BF16, tag="th",
                                           name=f"th{d}")
                    nc.scalar.activation(th[d][:], sg_next[d][:, 4 * CB:5 * CB],
                                         AF.Tanh, bias=neg1_t[:], scale=2.0)
                # DVE: h = so * tanh(c).
                for d in (0, 1):
                    nc.vector.tensor_tensor(
                        h_ts[d][:, hpos(d, s) * CB:(hpos(d, s) + 1) * CB],
                        sg_cur[d][:, 2 * CB:3 * CB], th[d][:], ALU.mult)
                    sg_cur[d] = sg_next[d]

        # keep the exp/ln table phase strictly after the sigmoid/tanh phase
        tc.no_sync_barrier()

        EMC = 512
        with (
            tc.tile_pool(name="emps", bufs=4, space="PSUM") as emps,
            tc.tile_pool(name="crf", bufs=4) as crf,
            tc.tile_pool(name="crfps", bufs=2, space="PSUM") as crfps,
        ):
            nchunk = T * BL // EMC
            emorder = []
            lo, hi = 0, nchunk - 1
            while lo <= hi:
                emorder.append(lo)
                if hi != lo:
                    emorder.append(hi)
                lo += 1
                hi -= 1
            RB = WARM * CB  # start of the real (non-warmup) region
            for ch in emorder:
                ep = emps.tile([K, EMC], F32, tag="ep")
                nc.tensor.matmul(ep[:], ckf_t[:],
                                 h_f[:, RB + ch * EMC:RB + (ch + 1) * EMC],
                                 start=True, stop=False)
                nc.tensor.matmul(ep[:], ckb_t[:],
                                 h_b[:, RB + ch * EMC:RB + (ch + 1) * EMC],
                                 start=False, stop=True)
                nc.scalar.activation(em_e[:, ch * EMC:(ch + 1) * EMC], ep[:],
                                     AF.Exp, bias=embias_t[:], scale=1.0)

            def ecol(tau):
                return (tau % (T // NCH)) * CB + (tau // (T // NCH)) * BL

            a_cur = crf.tile([K, BL], BF16, tag="a")
            nc.vector.tensor_copy(a_cur[:], em_e[:, ecol(0):ecol(0) + BL])
            b_cur = crf.tile([K, BL], BF16, tag="b")
            nc.vector.tensor_copy(b_cur[:], em_e[:, ecol(T - 1):ecol(T - 1) + BL])

            for s in range(1, MID + 1):
                aps = crfps.tile([K, BL], F32, tag="aps")
                nc.tensor.matmul(aps[:], ae_t[:], a_cur[:], start=True, stop=True)
                a_new = crf.tile([K, BL], BF16, tag="a")
                nc.vector.tensor_tensor(a_new[:], aps[:],
                                        em_e[:, ecol(s):ecol(s) + BL], ALU.mult)
                a_cur = a_new

                if s <= MID - 1:
                    t_b = T - 1 - s
                    bps = crfps.tile([K, BL], F32, tag="bps")
                    nc.tensor.matmul(bps[:], aet_t[:], b_cur[:], start=True, stop=True)
                    b_new = crf.tile([K, BL], BF16, tag="b")
                    if t_b == MID:
                        nc.vector.tensor_copy(b_new[:], bps[:])
                    else:
                        nc.vector.tensor_tensor(b_new[:], bps[:],
                                                em_e[:, ecol(t_b):ecol(t_b) + BL],
                                                ALU.mult)
                    b_cur = b_new

            prod = crf.tile([K, BL], F32, tag="prod")
            nc.vector.tensor_tensor(prod[:], a_cur[:], b_cur[:], ALU.mult)
            sps = crfps.tile([1, BL], F32, tag="aps")
            nc.tensor.matmul(sps[:], ones_t[:], prod[:], start=True, stop=True)
            logz = crf.tile([1, BL], F32, tag="logz")
            nc.scalar.activation(logz[:], sps[:], AF.Ln)
            logz2 = crf.tile([1, BL], F32, tag="logz2")
            nc.vector.tensor_scalar(logz2[:], logz[:], float(T * DELTA), None, ALU.add)
            nc.sync.dma_start(out[:], logz2[:])

    nc.compile()
    return nc


def _gate_permute(w):
    """Reorder gate blocks from reference (i,f,g,o) to kernel (i,f,o,g) and
    pre-double the g block so tanh(g) = 2*sigmoid(2g)-1 needs only sigmoid."""
    i, f, g, o = np.split(w, 4, axis=-1)
    return np.concatenate([i, f, o, 2.0 * g], axis=-1)


def _stage(tokens, emb, Wk_f, Wr_f, b_f, Wk_b, Wr_b, b_b, crf_kernel, crf_bias,
           trans):
    """Host staging: build the per-core input maps."""
    emb_aug = np.concatenate(
        [emb, np.ones((V, 1), np.float32), np.zeros((V, EA - E - 1), np.float32)], 1)
    wk_aug_f = np.concatenate([Wk_f, b_f[None], np.zeros((EA - E - 1, G4), np.float32)], 0)
    wk_aug_b = np.concatenate([Wk_b, b_b[None], np.zeros((EA - E - 1, G4), np.float32)], 0)
    Ae = np.exp(trans).astype(np.float32)

    shared = {
        "emb": emb_aug,
        "wk_f": np.ascontiguousarray(_gate_permute(wk_aug_f)).astype(NPBF16),
        "wk_b": np.ascontiguousarray(_gate_permute(wk_aug_b)).astype(NPBF16),
        "wr_f": np.ascontiguousarray(_gate_permute(Wr_f)).astype(NPBF16),
        "wr_b": np.ascontiguousarray(_gate_permute(Wr_b)).astype(NPBF16),
        "ck_f": np.ascontiguousarray(crf_kernel[:U]).astype(NPBF16),
        "ck_b": np.ascontiguousarray(crf_kernel[U:]).astype(NPBF16),
        "ae": np.ascontiguousarray(Ae).astype(NPBF16),
        "aet": np.ascontiguousarray(Ae.T).astype(NPBF16),
        "embias": (crf_bias - DELTA).astype(np.float32).reshape(K, 1),
        "ident": np.eye(128, dtype=np.float32),
    }

    NCH, WARM = 16, 8
    NP_ = T // NCH + WARM
    CL = T // NCH
    ss = np.arange(NP_)[:, None]
    jj = np.arange(NCH)[None, :]
    tf = np.clip(CL * jj - WARM + ss, 0, T - 1)           # [NP_, NCH] fwd times
    tb = np.clip(CL - 1 + WARM + CL * jj - ss, 0, T - 1)  # bwd (slot-relabeled)

    def tokmat(tc_, tm):
        full = tc_[:, tm].transpose(2, 0, 1).reshape(NCH * BL, NP_)  # [256, NP_]
        tk = np.empty((128, 2 * NP_), np.int32)
        tk[:, 0::2] = full[0:128]
        tk[:, 1::2] = full[128:256]
        return np.ascontiguousarray(tk)

    in_maps = []
    for c in range(NCORES):
        tc_ = tokens[c * BL:(c + 1) * BL].astype(np.int32)  # [16, T]
        in_maps.append({"tok_f": tokmat(tc_, tf),
                        "tok_b": tokmat(tc_, tb), **shared})
    return in_maps


_PROGRAM_CACHE = {}


def kernel(tokens, emb, Wk_f, Wr_f, b_f, Wk_b, Wr_b, b_b, crf_kernel, crf_bias, trans):
    from concourse.bass_utils import run_bass_kernel_spmd

    tokens = np.asarray(tokens)
    emb = np.asarray(emb, dtype=np.float32)
    Wk_f = np.asarray(Wk_f, np.float32); Wr_f = np.asarray(Wr_f, np.float32)
    Wk_b = np.asarray(Wk_b, np.float32); Wr_b = np.asarray(Wr_b, np.float32)
    b_f = np.asarray(b_f, np.float32); b_b = np.asarray(b_b, np.float32)
    crf_kernel = np.asarray(crf_kernel, np.float32)
    crf_bias = np.asarray(crf_bias, np.float32)
    trans = np.asarray(trans, np.float32)

    if "nc" not in _PROGRAM_CACHE:
        _PROGRAM_CACHE["nc"] = _build_program()
    nc = _PROGRAM_CACHE["nc"]

    in_maps = _stage(tokens, emb, Wk_f, Wr_f, b_f, Wk_b, Wr_b, b_b,
                     crf_kernel, crf_bias, trans)
    res = run_bass_kernel_spmd(nc, in_maps, core_ids=list(range(NCORES)))
    outs = [res.results[c]["out"].reshape(BL).astype(np.float32) for c in range(NCORES)]
    return np.concatenate(outs, axis=0)
